# revision 3
# baseline (speedup 1.0000x reference)
"""LogitSeparator Trainium2 kernel.

For each (b, d) of schemas (64, 32), left-align the zone
logits[b, start:end] (length = schemas[b,d] <= 255, zones tiling the row
contiguously) into out[b, d, :8192] with zero padding, plus the in-zone
boolean mask.

Since zone lengths are < 256, every output column >= 256 is structurally
zero/False.  The device computes the data-dependent part — a 256-wide
gathered+masked slab per (b, d) row and its mask — and the host unshards
the slabs into the zero canvas.

Sharding: pure data parallel, 8 batch rows per core.  Per core the 256
ragged (b, d) rows map onto 2 x 128 SBUF partitions (row r = h*128 + p).

Device pipeline (per core), ordered for latency: a single aux DMA lands
gather offsets + zone lens + iota in SBUF; gpsimd runs one indirect
gather per half (the ragged gather proper); DVE builds the j < len mask
and multiplies each gathered half as soon as that half's DMA completes;
the halves stream out on two different HWDGE engines (sync/scalar) so
the first write's ~3us trigger-to-data latency overlaps the second
half's mask-multiply.  Engine blocks stay open (and gpsimd's expensive
dge_drain is skipped) until the writes land so end-of-block DGE drains
don't stuff the DMA rings under the output transfers.
"""

import numpy as np

import concourse.bass as bass
import concourse.mybir as mybir
from concourse.bass_utils import run_bass_kernel_spmd

B, D, L = 64, 32, 8192
NCORES = 8
BPC = B // NCORES           # batch rows per core
R = BPC * D                 # ragged rows per core (256)
P = 128                     # SBUF partitions
HALVES = R // P             # 2
SLAB = 256                  # max zone length (schemas < 256)
NPAD = BPC * L + SLAB       # padded flat logits length per core
W = HALVES * SLAB           # 512

# aux layout (int32): [0:2] gather flat starts per half, [2:4] zone lens
# per half, [4:260] iota 0..255.
AUXW = 4 + SLAB

# Split the out-slab write per half across sync/scalar (overlaps the
# first write's HWDGE latency with the second half's mul + write).
SPLIT_OUT = False

_NC_CACHE = {}


def build_nc():
    nc = bass.Bass()
    lg = nc.declare_dram_parameter(
        "logits_flat", [NPAD, 1], mybir.dt.float32, isOutput=False
    )
    aux = nc.declare_dram_parameter("aux", [P, AUXW], mybir.dt.int32, isOutput=False)
    out = nc.declare_dram_parameter(
        "out", [R * SLAB, 1], mybir.dt.float32, isOutput=True
    )
    msk = nc.declare_dram_parameter(
        "mask", [R * SLAB, 1], mybir.dt.uint8, isOutput=True
    )

    # Raw bass engine blocks (no Tile): the schedule is a short, explicit
    # latency chain and Tile's drain aggregation gets in the way.
    with (
        nc.sbuf_tensor([P, AUXW], mybir.dt.int32) as aux_t,
        nc.sbuf_tensor([P, W], mybir.dt.float32) as gat2,
        nc.sbuf_tensor([P, W], mybir.dt.float32) as maskf2,
        nc.sbuf_tensor([P, W], mybir.dt.uint8) as slabm2,
        nc.semaphore("asem") as asem,   # aux input DMA completion
        nc.semaphore("gsem0") as gsem0,  # gather half-0 DMA completion
        nc.semaphore("gsem1") as gsem1,  # gather half-1 DMA completion
        nc.semaphore("vsem") as vsem,   # DVE milestones
        nc.semaphore("dsem") as dsem,   # output DMA completions
        nc.Block(no_gpsimd_drain=True) as block,
    ):
        iota_ap = aux_t[:, 4 : 4 + SLAB]
        # DRAM [R*SLAB, 1] viewed [p, h, j]: flat = (h*128+p)*256 + j,
        # i.e. ragged row r = h*128+p owns the 256-element slab at r*256.
        out3 = out.rearrange("(h p j) one -> p h (j one)", h=HALVES, p=P, j=SLAB)
        msk3 = msk.rearrange("(h p j) one -> p h (j one)", h=HALVES, p=P, j=SLAB)
        DTOT = 48 if SPLIT_OUT else 32

        @block.sync
        def _(sync):
            sync.dma_start(out=aux_t[:], in_=aux[:]).then_inc(asem, 16)
            if SPLIT_OUT:
                sync.wait_ge(vsem, 3)  # mul of half 0 done
                sync.dma_start(
                    out=out3[:, 0:1, :], in_=gat2[:, 0:SLAB].unsqueeze(1)
                ).then_inc(dsem, 16)
            else:
                sync.wait_ge(vsem, 4)  # both muls done
                sync.dma_start(
                    out=out3[:], in_=gat2[:].rearrange("p (h j) -> p h j", h=HALVES)
                ).then_inc(dsem, 16)
            # all output DMAs landed before the kernel ends
            sync.wait_ge(dsem, DTOT)

        @block.scalar
        def _(scalar):
            scalar.wait_ge(vsem, 2)  # u8 mask cast done
            scalar.dma_start(
                out=msk3[:], in_=slabm2[:].rearrange("p (h j) -> p h j", h=HALVES)
            ).then_inc(dsem, 16)
            if SPLIT_OUT:
                scalar.wait_ge(vsem, 4)  # mul of half 1 done
                scalar.dma_start(
                    out=out3[:, 1:2, :], in_=gat2[:, SLAB:W].unsqueeze(1)
                ).then_inc(dsem, 16)
            # hold the block open: scalar's end-of-block HWDGE drain would
            # otherwise stuff the DMA rings under the final out write
            scalar.wait_ge(dsem, DTOT)

        @block.gpsimd
        def _(gp):
            gp.wait_ge(asem, 16)  # gather offsets in SBUF
            # The ragged gather: one 128-descriptor indirect DMA per half.
            # Each half incs its own semaphore — the two DMAs complete in
            # nondeterministic order across the rings, so a shared counter
            # cannot tell WHICH half landed.
            for h, gs in ((0, gsem0), (1, gsem1)):
                gp.indirect_dma_start(
                    out=gat2[:, h * SLAB : (h + 1) * SLAB],
                    out_offset=None,
                    in_=lg[:],
                    in_offset=bass.IndirectOffsetOnAxis(
                        ap=aux_t[:, h : h + 1], axis=0
                    ),
                ).then_inc(gs, 16)

        @block.vector
        def _(v):
            v.wait_ge(asem, 16)  # lens + iota in SBUF
            # mask[p, h, j] = j < len_ph  (int32 compare, f32 0/1 out)
            for h in range(HALVES):
                v.tensor_tensor(
                    out=maskf2[:, h * SLAB : (h + 1) * SLAB],
                    in0=iota_ap,
                    in1=aux_t[:, 2 + h : 3 + h].to_broadcast([P, SLAB]),
                    op=mybir.AluOpType.is_lt,
                ).then_inc(vsem, 1 if h == HALVES - 1 else 0)
            v.drain()  # flush DVE pipeline: maskf2 RAW below
            v.tensor_copy(out=slabm2[:], in_=maskf2[:]).then_inc(vsem, 1)
            # zero the gathered tail garbage per half as soon as that
            # half's gather DMA lands
            for h, gs in ((0, gsem0), (1, gsem1)):
                v.wait_ge(gs, 16)
                v.tensor_mul(
                    out=gat2[:, h * SLAB : (h + 1) * SLAB],
                    in0=gat2[:, h * SLAB : (h + 1) * SLAB],
                    in1=maskf2[:, h * SLAB : (h + 1) * SLAB],
                ).then_inc(vsem, 1)
    return nc


def make_in_maps(schemas, logits):
    """Shard full inputs into per-core input maps for the SPMD kernel."""
    sch = np.asarray(schemas).astype(np.int64)
    lg = np.ascontiguousarray(np.asarray(logits, dtype=np.float32))
    cs = np.cumsum(sch, axis=1)
    start = cs - sch                     # (B, D) zone starts

    in_maps = []
    for c in range(NCORES):
        b0 = c * BPC
        flat = np.concatenate(
            [lg[b0 : b0 + BPC].reshape(-1), np.zeros(SLAB, np.float32)]
        ).reshape(NPAD, 1)
        gflat = (
            np.arange(BPC, dtype=np.int64)[:, None] * L + start[b0 : b0 + BPC]
        ).reshape(R)
        aux = np.empty((P, AUXW), dtype=np.int32)
        # ragged row r = h*128 + p  ->  aux[p, h]
        aux[:, 0:2] = gflat.reshape(HALVES, P).T
        aux[:, 2:4] = (
            sch[b0 : b0 + BPC].reshape(R).reshape(HALVES, P).T.astype(np.int32)
        )
        aux[:, 4:] = np.arange(SLAB, dtype=np.int32)
        in_maps.append({"logits_flat": flat, "aux": aux})
    return in_maps


def assemble(results):
    """Unshard per-core slab outputs into the full-shape zero canvas."""
    out = np.zeros((B, D, L), dtype=np.float32)
    msk = np.zeros((B, D, L), dtype=np.bool_)
    for c in range(NCORES):
        b0 = c * BPC
        o = np.asarray(results[c]["out"]).reshape(BPC, D, SLAB)
        m = np.asarray(results[c]["mask"]).reshape(BPC, D, SLAB)
        out[b0 : b0 + BPC, :, :SLAB] = o
        msk[b0 : b0 + BPC, :, :SLAB] = m.astype(bool)
    return out, msk


def kernel(schemas, logits):
    in_maps = make_in_maps(schemas, logits)
    if "nc" not in _NC_CACHE:
        _NC_CACHE["nc"] = build_nc()
    res = run_bass_kernel_spmd(_NC_CACHE["nc"], in_maps, list(range(NCORES))).results
    return assemble(res)
